# revision 27
# baseline (speedup 1.0000x reference)
"""LCNN (lattice GNN) Trainium2 kernel — 8-core SPMD, v2.

Strategy (v2):
  - Shard the N=100000 nodes across 8 cores (12500 each = 98 tiles of 128).
  - Node-feature tables packed 4 nodes per 256B row (dma_gather's minimum
    element size), so the gather row index = node>>2 fits in int16 without
    table segmentation (25000 rows < 32768).  x table: [25000, 64] f32
    (slot s = cols 16s..16s+2).  h1 table: [25000, 128] bf16
    (slot s = cols 32s..32s+18).
  - Per 128-node tile, 15 InstDMAGatherAnt issues (chunks of <=1024 idx —
    larger issues wedge the device, bisected on HW) gather all 14592
    (=P*K*128) neighbor rows (vs 114 x 1us-overhead indirect DMAs in v1).
    A 4-way mask-select on DVE (masks from slot = node&3, shipped 2-bit
    packed) extracts the right slot.
  - The conv weights are shared across the P=6 permutations and BN is an
    affine, so sum the selected features over p on DVE first; then only
    1 (block1) / 3 (block2) PE transpose+matmul pairs per tile.
  - Block1 in f32; h1 stored bf16; block2 matmul in bf16 (f32 accum).
  - AllGather the packed x shards and packed h1 shards ([3125, 256B] each).
  - Tail per tile: Wc matmul -> LayerNorm -> softplus -> masked accumulate;
    per-core [25] feature sums; the tiny head finishes on host in fp32.
  - jax persistent compilation cache: the first run_bass_kernel_spmd call
    compiles + caches the executable; later calls (fresh jit closures in
    run_bass_via_pjrt) hit the disk cache instead of recompiling.
"""

import sys

sys.path.insert(0, "/opt/trn_rl_repo")

import numpy as np
import jax

jax.config.update("jax_compilation_cache_dir", "/tmp/jax_cache_lcnn")
jax.config.update("jax_persistent_cache_min_entry_size_bytes", -1)
jax.config.update("jax_persistent_cache_min_compile_time_secs", 0)

from concourse import bacc, mybir
import concourse.bass as bass
import concourse.tile as tile
from concourse import bass_utils
from concourse.masks import make_identity

# Problem constants (hardcoded per contract)
N, P, K = 100000, 6, 19
F0, F, SF = 3, 19, 25
NC = 8
SHARD = N // NC          # 12500
NT = 98                  # tiles of 128 rows
SHARD_PAD = NT * 128     # 12544
PK = P * K               # 114
NI = PK * 128            # 14592 gather indices per tile
NIW = NI // 16           # 912 wrapped idx columns
XROWS = N // 4           # 25000 packed table rows
SROWS = SHARD // 4       # 3125 packed rows per shard
SROWS_PAD = SHARD_PAD // 4  # 3136
BN_EPS = 1e-5
LN_EPS = 1e-5
LOG2 = 0.6931

F32 = mybir.dt.float32
BF16 = mybir.dt.bfloat16
I16 = mybir.dt.int16
U8 = mybir.dt.uint8
PKB = (PK + 3) // 4      # 29 packed slot bytes (4 x 2-bit slots each)

ADD = mybir.AluOpType.add
MULT = mybir.AluOpType.mult

# cst packed-constant columns
C_A1, C_D1, C_A2, C_D2 = 0, 19, 38, 57
C_BC, C_LNG, C_LNB = 76, 101, 126
C_MASK, C_ONES = 151, 152
C_ZERO, C_EPS = 153, 154
C_TOT = 155


def build_nc():
    nc = bacc.Bacc("TRN2", target_bir_lowering=False, debug=False,
                   num_devices=NC)

    x_t = nc.dram_tensor("x", [SHARD, F0], F32, kind="ExternalInput")
    row_t = nc.dram_tensor("row", [NT, 16, NIW], I16, kind="ExternalInput")
    slot_t = nc.dram_tensor("slot", [SHARD_PAD, PKB], U8, kind="ExternalInput")
    w1_t = nc.dram_tensor("w1", [F0 * K, F], F32, kind="ExternalInput")
    w2_t = nc.dram_tensor("w2", [F * K, F], F32, kind="ExternalInput")
    wc_t = nc.dram_tensor("wc", [F, SF], F32, kind="ExternalInput")
    cst_t = nc.dram_tensor("cst", [128, C_TOT], F32, kind="ExternalInput")
    out_t = nc.dram_tensor("out", [SF, 1], F32, kind="ExternalOutput")

    with tile.TileContext(nc) as tc:
        with (
            tc.tile_pool(name="const", bufs=1) as cpool,
            tc.tile_pool(name="idxp", bufs=2) as ipool,
            tc.tile_pool(name="gath", bufs=3) as gpool,
            tc.tile_pool(name="sel", bufs=2) as spool,
            tc.tile_pool(name="work", bufs=3) as wpool,
            tc.tile_pool(name="dx", bufs=1, space="DRAM") as dxp,
            tc.tile_pool(name="dh", bufs=1, space="DRAM") as dhp,
            tc.tile_pool(name="pst", bufs=3, space="PSUM") as pst,
            tc.tile_pool(name="psa", bufs=2, space="PSUM") as psa,
        ):
            # ---- constants ----
            cst = cpool.tile([128, C_TOT], F32)
            nc.sync.dma_start(cst[:], cst_t[:, :])
            w1s = cpool.tile([F0 * K, F], F32)
            nc.sync.dma_start(w1s[:], w1_t[:, :])
            w2f = cpool.tile([128, 3, F], F32)
            nc.vector.memset(w2f[:], 0.0)
            nc.sync.dma_start(w2f[:, 0, :], w2_t[0:128, :])
            nc.sync.dma_start(w2f[:, 1, :], w2_t[128:256, :])
            nc.sync.dma_start(w2f[:105, 2, :], w2_t[256:361, :])
            w2b = cpool.tile([128, 3, F], BF16)
            nc.vector.tensor_copy(w2b[:], w2f[:])
            wcs = cpool.tile([F, SF], F32)
            nc.sync.dma_start(wcs[:], wc_t[:, :])
            ident = cpool.tile([128, 128], F32)
            make_identity(nc, ident[:])
            slot_sb = cpool.tile([128, NT, PKB], U8)
            nc.sync.dma_start(slot_sb[:],
                              slot_t.rearrange("(t p) k -> p t k", p=128))

            # ---- DRAM node tables (4 nodes per 256B row) ----
            # pack this core's raw x shard into 4-slot rows (single DMA; pad
            # lanes stay garbage — the slot select never reads them).
            # Collectives can't read IO tensors, so xp_shard is a DRAM tile.
            xp_shard = dxp.tile([SROWS, 64], F32)
            xp_full = dxp.tile([XROWS, 64], F32)
            h1p_shard = dhp.tile([SROWS_PAD, 128], BF16)
            h1p_full = dhp.tile([XROWS, 128], BF16)

            nc.sync.dma_start(
                xp_shard[:, :].rearrange("r (s f) -> r s f", s=4)[:, :, 0:F0],
                x_t[:, :].rearrange("(r s) f -> r s f", s=4))
            nc.gpsimd.collective_compute(
                "AllGather", mybir.AluOpType.bypass,
                replica_groups=[list(range(NC))],
                ins=[xp_shard[:, :].opt()],
                outs=[xp_full[:, :].opt()],
            )

            acc = cpool.tile([128, SF], F32)
            nc.vector.memset(acc[:], 0.0)

            def load_idx(t):
                idx = ipool.tile([128, NIW], I16, tag="idx")
                nc.sync.dma_start(
                    idx[:],
                    row_t[t:t + 1, :, :].broadcast_to((8, 16, NIW)))
                return idx

            # HW wedges on dma_gather with >1024 indices per issue, and
            # intermittently near that bound (NRT_EXEC_UNIT_UNRECOVERABLE,
            # bisected 2026-08-09) — chunk with 2x margin.
            GCH = 512
            GCHUNKS = [(q * GCH, min(GCH, NI - q * GCH))
                       for q in range((NI + GCH - 1) // GCH)]

            def gather_tile(g, table, idx, elem):
                for (i0, n_q) in GCHUNKS:
                    nc.gpsimd.dma_gather(
                        out_ap=g[:, i0 // 128:(i0 + n_q) // 128, :],
                        in_ap=table[:, :],
                        idxs_ap=idx[:, i0 // 16:(i0 + n_q) // 16],
                        num_idxs=n_q, num_idxs_reg=n_q, elem_size=elem)

            def make_masks(t, dt_):
                # unpack 4 x 2-bit slots per byte: slot j=4b+q at bits 2q
                # (bit ops can't cast, so unpack in uint8; is_equal casts)
                slotu = spool.tile([128, PKB, 4], U8, tag="slotu")
                for q in range(4):
                    nc.vector.tensor_scalar(
                        slotu[:, :, q], slot_sb[:, t, :], 2 * q, 3,
                        mybir.AluOpType.logical_shift_right,
                        mybir.AluOpType.bitwise_and)
                sl = slotu[:].rearrange("a b q -> a (b q)")[:, 0:PK]
                ms = []
                for s in range(4):
                    m = spool.tile([128, PK], dt_, tag=f"m{s}")
                    nc.vector.tensor_scalar(
                        m[:], sl, float(s), None,
                        mybir.AluOpType.is_equal)
                    ms.append(m)
                return ms

            # ---- block 1: h1 = A1*(sum_p X_p)@W1 + D1 ----
            for t in range(NT):
                idx = load_idx(t)
                ms = make_masks(t, F32)
                g1 = gpool.tile([128, PK, 64], F32, tag="g")
                gather_tile(g1, xp_full, idx, 64)
                X = spool.tile([128, PK, F0], F32, tag="X")
                tmp = spool.tile([128, PK, F0], F32, tag="tmp")
                for s in range(4):
                    tgt = X if s == 0 else tmp
                    nc.vector.tensor_tensor(
                        out=tgt[:], in0=g1[:, :, 16 * s:16 * s + F0],
                        in1=ms[s][:].unsqueeze(2).broadcast_to((128, PK, F0)),
                        op=MULT)
                    if s:
                        nc.vector.tensor_tensor(
                            out=X[:], in0=X[:], in1=tmp[:], op=ADD)
                Xp = X[:].rearrange("a (p k) f -> a p (k f)", p=P)
                Y = wpool.tile([128, K * F0], F32, tag="Y1")
                nc.vector.tensor_copy(Y[:], Xp[:, 0, :])
                for p in range(1, P):
                    nc.vector.tensor_tensor(
                        out=Y[:], in0=Y[:], in1=Xp[:, p, :], op=ADD)
                tp = pst.tile([128, 128], F32, tag="tp")
                nc.tensor.transpose(out=tp[:K * F0, :], in_=Y[:],
                                    identity=ident[:])
                lh = wpool.tile([K * F0, 128], F32, tag="lh1")
                nc.vector.tensor_copy(lh[:], tp[:K * F0, :])
                psh = psa.tile([128, F], F32, tag="psh")
                nc.tensor.matmul(out=psh[:], lhsT=lh[:], rhs=w1s[:],
                                 start=True, stop=True)
                s1 = wpool.tile([128, F], F32, tag="s1")
                nc.vector.tensor_tensor(
                    out=s1[:], in0=psh[:], in1=cst[:, C_A1:C_A1 + F], op=MULT)
                nc.vector.tensor_tensor(
                    out=s1[:], in0=s1[:], in1=cst[:, C_D1:C_D1 + F], op=ADD)
                s1b = wpool.tile([128, 32], BF16, tag="s1b")
                nc.vector.memset(s1b[:, F:32], 0.0)
                nc.vector.tensor_copy(s1b[:, 0:F], s1[:])
                # packed write: nodes 128t..128t+127 -> rows 32t..32t+31
                nc.sync.dma_start(
                    h1p_shard[32 * t:32 * t + 32, :]
                        .rearrange("r (s f) -> r s f", s=4),
                    s1b[:])

            # ---- AllGather packed h1 shards -> full table ----
            nc.gpsimd.collective_compute(
                "AllGather", mybir.AluOpType.bypass,
                replica_groups=[list(range(NC))],
                ins=[h1p_shard[0:SROWS, :].opt()],
                outs=[h1p_full[:, :].opt()],
            )

            # ---- block 2 + head ----
            for t in range(NT):
                idx = load_idx(t)
                ms = make_masks(t, BF16)
                g2 = gpool.tile([128, PK, 128], BF16, tag="g")
                gather_tile(g2, h1p_full, idx, 128)
                X2 = spool.tile([128, PK, F], BF16, tag="X")
                tm2 = spool.tile([128, PK, F], BF16, tag="tmp")
                for s in range(4):
                    tgt = X2 if s == 0 else tm2
                    nc.vector.tensor_tensor(
                        out=tgt[:], in0=g2[:, :, 32 * s:32 * s + F],
                        in1=ms[s][:].unsqueeze(2).broadcast_to((128, PK, F)),
                        op=MULT)
                    if s:
                        nc.vector.tensor_tensor(
                            out=X2[:], in0=X2[:], in1=tm2[:], op=ADD)
                X2p = X2[:].rearrange("a (p k) f -> a p (k f)", p=P)
                Y2 = wpool.tile([128, K * F], F32, tag="Y2")
                nc.vector.tensor_copy(Y2[:], X2p[:, 0, :])
                for p in range(1, P):
                    nc.vector.tensor_tensor(
                        out=Y2[:], in0=Y2[:], in1=X2p[:, p, :], op=ADD)
                ps2 = psa.tile([128, F], F32, tag="psh")
                for c in range(3):
                    rows = 128 if c < 2 else 105
                    tp2 = pst.tile([128, 128], F32, tag="tp")
                    nc.tensor.transpose(out=tp2[:rows, :],
                                        in_=Y2[:, 128 * c:128 * c + rows],
                                        identity=ident[:])
                    lh2 = wpool.tile([128, 128], BF16, tag="lh2")
                    nc.vector.tensor_copy(lh2[:rows, :], tp2[:rows, :])
                    nc.tensor.matmul(out=ps2[:], lhsT=lh2[:rows, :],
                                     rhs=w2b[:rows, c, :],
                                     start=(c == 0), stop=(c == 2))
                s2 = wpool.tile([128, F], F32, tag="s2")
                nc.vector.tensor_tensor(
                    out=s2[:], in0=ps2[:], in1=cst[:, C_A2:C_A2 + F], op=MULT)
                nc.vector.tensor_tensor(
                    out=s2[:], in0=s2[:], in1=cst[:, C_D2:C_D2 + F], op=ADD)
                # h2 @ Wc
                tp3 = pst.tile([F, 128], F32, tag="tp")
                nc.tensor.transpose(out=tp3[:], in_=s2[:], identity=ident[:])
                h2T = wpool.tile([F, 128], F32, tag="h2T")
                nc.vector.tensor_copy(h2T[:], tp3[:])
                ps3 = psa.tile([128, SF], F32, tag="ps3")
                nc.tensor.matmul(out=ps3[:], lhsT=h2T[:], rhs=wcs[:],
                                 start=True, stop=True)
                h3 = wpool.tile([128, SF], F32, tag="h3")
                nc.vector.tensor_tensor(
                    out=h3[:], in0=ps3[:], in1=cst[:, C_BC:C_BC + SF], op=ADD)
                # LayerNorm over SF
                mu = wpool.tile([128, 1], F32, tag="mu")
                nc.vector.tensor_reduce(
                    out=mu[:], in_=h3[:], axis=mybir.AxisListType.X, op=ADD)
                nc.scalar.mul(mu[:], mu[:], 1.0 / SF)
                xc = wpool.tile([128, SF], F32, tag="xc")
                nc.vector.tensor_scalar_sub(xc[:], h3[:], mu[:])
                sq = wpool.tile([128, SF], F32, tag="sq")
                var = wpool.tile([128, 1], F32, tag="var")
                nc.scalar.activation(
                    out=sq[:], in_=xc[:],
                    func=mybir.ActivationFunctionType.Square,
                    bias=cst[:, C_ZERO:C_ZERO + 1],
                    accum_out=var[:])
                lnv = wpool.tile([128, 1], F32, tag="lnv")
                nc.scalar.activation(
                    out=lnv[:], in_=var[:],
                    func=mybir.ActivationFunctionType.Ln,
                    bias=cst[:, C_EPS:C_EPS + 1], scale=1.0 / SF)
                rstd = wpool.tile([128, 1], F32, tag="rstd")
                nc.scalar.activation(
                    out=rstd[:], in_=lnv[:],
                    func=mybir.ActivationFunctionType.Exp,
                    bias=cst[:, C_ZERO:C_ZERO + 1], scale=-0.5)
                y = wpool.tile([128, SF], F32, tag="y")
                nc.vector.tensor_scalar_mul(y[:], xc[:], rstd[:])
                nc.vector.tensor_tensor(
                    out=y[:], in0=y[:], in1=cst[:, C_LNG:C_LNG + SF], op=MULT)
                nc.vector.tensor_tensor(
                    out=y[:], in0=y[:], in1=cst[:, C_LNB:C_LNB + SF], op=ADD)
                ey = wpool.tile([128, SF], F32, tag="ey")
                nc.scalar.activation(
                    out=ey[:], in_=y[:],
                    func=mybir.ActivationFunctionType.Exp,
                    bias=cst[:, C_ZERO:C_ZERO + 1])
                sp = wpool.tile([128, SF], F32, tag="sp")
                nc.scalar.activation(
                    out=sp[:], in_=ey[:],
                    func=mybir.ActivationFunctionType.Ln,
                    bias=cst[:, C_ONES:C_ONES + 1])
                if t == NT - 1:
                    nc.vector.tensor_scalar_mul(
                        sp[:], sp[:], cst[:, C_MASK:C_MASK + 1])
                nc.vector.tensor_tensor(
                    out=acc[:], in0=acc[:], in1=sp[:], op=ADD)

            # ---- per-core feature sums: [25,1] = acc.T @ ones ----
            ps4 = psa.tile([SF, 1], F32, tag="ps3")
            nc.tensor.matmul(out=ps4[:], lhsT=acc[:],
                             rhs=cst[:, C_ONES:C_ONES + 1],
                             start=True, stop=True)
            res = wpool.tile([SF, 1], F32, tag="res")
            nc.scalar.copy(res[:], ps4[:])
            nc.sync.dma_start(out_t[:, :], res[:])

    nc.compile()
    return nc


_NC_CACHE = None


def _get_nc():
    global _NC_CACHE
    if _NC_CACHE is None:
        _NC_CACHE = build_nc()
        # the jit lowering serializes the BIR on every run_bass_kernel_spmd
        # call; the module is frozen after build, so memoize the bytes
        jb = _NC_CACHE.to_json_bytes()
        _NC_CACHE.to_json_bytes = lambda: jb
    return _NC_CACHE


def _make_in_maps(inputs):
    x = np.ascontiguousarray(inputs["x"], dtype=np.float32)
    nbr = np.ascontiguousarray(inputs["nbr_idx"], dtype=np.int32)

    def fold(g, be, rm, rv, b):
        a = g / np.sqrt(rv + BN_EPS)
        d = P * (a * (b - rm) + be)
        return a.astype(np.float32), d.astype(np.float32)

    a1, d1 = fold(inputs["g1"], inputs["be1"], inputs["rm1"], inputs["rv1"],
                  inputs["b1"])
    a2, d2 = fold(inputs["g2"], inputs["be2"], inputs["rm2"], inputs["rv2"],
                  inputs["b2"])

    cst = np.zeros((128, C_TOT), np.float32)
    cst[:, C_A1:C_A1 + F] = a1
    cst[:, C_D1:C_D1 + F] = d1
    cst[:, C_A2:C_A2 + F] = a2
    cst[:, C_D2:C_D2 + F] = d2
    cst[:, C_BC:C_BC + SF] = inputs["bc"]
    cst[:, C_LNG:C_LNG + SF] = inputs["lng"]
    cst[:, C_LNB:C_LNB + SF] = inputs["lnb"]
    # last tile holds rows 97*128 .. 97*128+127; rows >= 12500-97*128=84 are pad
    cst[:84, C_MASK] = 1.0
    cst[:, C_ONES] = 1.0
    cst[:, C_EPS] = LN_EPS

    w1 = np.ascontiguousarray(inputs["W1"], np.float32)
    w2 = np.ascontiguousarray(inputs["W2"], np.float32)
    wc = np.ascontiguousarray(inputs["Wc"], np.float32)

    in_maps = []
    for c in range(NC):
        g = nbr[c * SHARD:(c + 1) * SHARD].reshape(SHARD, PK)
        gpad = np.zeros((SHARD_PAD, PK), np.int32)
        gpad[:SHARD] = g
        row = (gpad >> 2).astype(np.int16)
        slot = (gpad & 3).astype(np.uint8)
        # pack 4 x 2-bit slots per byte (slot j=4b+q at bits 2q of byte b)
        slot4 = np.zeros((SHARD_PAD, PKB * 4), np.uint8)
        slot4[:, :PK] = slot
        s4 = slot4.reshape(SHARD_PAD, PKB, 4)
        slotp = (s4[:, :, 0] | (s4[:, :, 1] << 2) | (s4[:, :, 2] << 4)
                 | (s4[:, :, 3] << 6)).astype(np.uint8)
        r3 = row.reshape(NT, 128, PK)                     # [t, n, j]
        ri = r3.transpose(0, 2, 1).reshape(NT, NI)        # [t, j*128+n]
        row16 = np.ascontiguousarray(
            ri.reshape(NT, NIW, 16).transpose(0, 2, 1))   # [t, i%16, i//16]
        in_maps.append({
            "x": np.ascontiguousarray(x[c * SHARD:(c + 1) * SHARD]),
            "row": row16, "slot": np.ascontiguousarray(slotp),
            "w1": w1, "w2": w2, "wc": wc, "cst": cst,
        })
    return in_maps


def kernel(trace=False, **inputs):
    import time as _time
    nc = _get_nc()
    in_maps = _make_in_maps(inputs)
    res = bass_utils.run_bass_kernel_spmd(
        nc, in_maps, core_ids=list(range(NC)), trace=False)
    if trace:
        t0 = _time.perf_counter()
        res = bass_utils.run_bass_kernel_spmd(
            nc, in_maps, core_ids=list(range(NC)), trace=False)
        kernel.last_wall_ns = (_time.perf_counter() - t0) * 1e9
    sums = np.stack([r["out"].reshape(SF) for r in res.results])  # [NC, SF]
    total = sums.sum(axis=0, dtype=np.float64).astype(np.float32)
    # finish head on host: h3 sums -> mean -> Wl -> Wf
    h3_sum = total - np.float32(N * LOG2)
    g = (h3_sum / np.float32(N)) @ inputs["Wl"] + inputs["bl"]
    out = g @ inputs["Wf"] + inputs["bf"]
    if trace:
        kernel.last_exec_time_ns = res.exec_time_ns
        kernel.last_results = res
    return out.astype(np.float32)
